# revision 27
# baseline (speedup 1.0000x reference)
"""Ernie4.5 attention layer on 8 Trainium2 NeuronCores.

Sharding (tensor-parallel over heads, 8-way):
  core i owns query heads {2i, 2i+1} and kv head i//2 (kv groups stay
  aligned with their query heads; kv projections are duplicated on the
  two cores sharing a kv head).
  Every core processes all B*S = 4096 tokens through projections + RoPE +
  attention for its own heads, producing attn^T [256 f, 4096 tok].
  Per-head 8-core AllToAlls redistribute head-slices -> token-slices,
  after which core i holds attn^T [2048 f, 512 tok] for global tokens
  [512*i, 512*(i+1)) and computes the o_proj rows for those tokens.
  Host-side gather is a pure concatenation.

Numerics: all matmul inputs are bf16 (x, weights, rotated Q/K, softmax
probabilities); every contraction accumulates in fp32 PSUM. End-to-end
max relative error vs the fp32 reference is ~3e-3.

Performance notes:
  - scores are computed transposed (S^T[k, q]) so exp(S) lands in SBUF
    already k-major for the P@V matmul; V is augmented with a ones
    column so the softmax denominator falls out of the same matmul.
  - causal structure: fully-masked key blocks are skipped, the diagonal
    band is column-trimmed, and the in-band triangle is zeroed with a
    0/1 mask multiply after exp.
  - DMA instruction count is minimized (one ~625ns HWDGE slot per
    dma_start): x / Wo / attn loads move 16 128-row slices in a single
    3D-strided DMA.
"""

import math
from contextlib import ExitStack

import numpy as np
import ml_dtypes

import concourse.bass as bass
import concourse.tile as tile
from concourse import bacc, mybir
from concourse.bass_utils import run_bass_kernel_spmd
from concourse.masks import make_identity

HIDDEN = 2048
N_HEADS = 16
N_KV_HEADS = 4
HEAD_DIM = 128
ROPE_BASE = 10000.0
B, S = 2, 2048
T = B * S                    # 4096 global tokens (batch-major)
N_CORES = 8
HEADS_PER_CORE = 2
P = 128
SCALE = 1.0 / math.sqrt(HEAD_DIM)

F32 = mybir.dt.float32
BF16 = mybir.dt.bfloat16
EXP = mybir.ActivationFunctionType.Exp

HS = HIDDEN // P             # 16 hidden slices
TB = T // 512                # 8 token blocks of 512
KB = T // P                  # 32 key blocks of 128 (16 per batch)
QC_PER_BATCH = S // P        # 16 query chunks of 128 per batch

SWAP_MASK = [i ^ 1 for i in range(32)]  # pair swap within 32-partition groups


class Ctx:
    """Pools + persistent tiles for one repetition of the program."""

    def __init__(self, tc, es, io, single_core=False):
        nc = tc.nc
        self.tc, self.nc, self.io = tc, nc, io
        self.single_core = single_core
        pool = lambda name, bufs, **kw: es.enter_context(tc.tile_pool(name=name, bufs=bufs, **kw))
        self.const = pool("const", 1)
        self.wp = pool("w", 1)
        self.xtp = pool("xt", 2)
        self.qtp = pool("qt", 1)
        self.ropep = pool("rope", 3)
        self.vaugp = pool("vaug", 1)
        self.expp = pool("expp", 9)          # [128,1024] bf16 pair tiles
        self.smallp = pool("small_sb", 3)
        self.otp = pool("ot_stage", 2)
        self.attntp = pool("attnt", 1)
        self.wop = pool("wo", 3)
        self.outp = pool("outsb", 2)
        self.dram = pool("dram", 1, space="DRAM")

        # weights first on the scalar queue (first matmul needs them);
        # one 3D-strided DMA per tensor: [hid, c] -> [128, hs, c]
        self.wq_sb = self.wp.tile([P, HS * HEADS_PER_CORE * HEAD_DIM], BF16, tag="wq", name="wq")
        self.wk_sb = self.wp.tile([P, HS * HEAD_DIM], BF16, tag="wk", name="wk")
        self.wv_sb = self.wp.tile([P, HS * HEAD_DIM], BF16, tag="wv", name="wv")
        for dst, src in ((self.wq_sb, io["wqT"]), (self.wk_sb, io["wkT"]), (self.wv_sb, io["wvT"])):
            nc.scalar.dma_start(
                out=dst[:].rearrange("p (h c) -> p h c", h=HS),
                in_=src[:].rearrange("(h p) c -> p h c", p=P),
            )

        self.identity = self.const.tile([P, P], BF16, tag="identity", name="identity")
        make_identity(nc, self.identity[:])
        self.tri_sb = self.const.tile([P, P], BF16, tag="tri", name="tri")
        nc.scalar.dma_start(out=self.tri_sb[:], in_=io["trimask"][:])
        self.cos_sb = self.const.tile([P, S], F32, tag="cos", name="cos")
        nc.scalar.dma_start(out=self.cos_sb[:], in_=io["cosT"][:])
        self.sin_sb = self.const.tile([P, S], F32, tag="sin", name="sin")
        nc.scalar.dma_start(out=self.sin_sb[:], in_=io["sinTs"][:])

        # persistent rotated Q^T (per head) and K^T, all [128, 4096] bf16
        self.QT = [self.qtp.tile([P, T], BF16, tag=f"QT{i}", name=f"QT{i}") for i in range(HEADS_PER_CORE)]
        self.KT = self.qtp.tile([P, T], BF16, tag="KT", name="KT")
        # V_aug[kb]: [128 tok, 128 d + ones column] bf16
        self.Vaug = [self.vaugp.tile([P, HEAD_DIM + 1], BF16, tag=f"va{k}", name=f"va{k}") for k in range(KB)]

        # per-head a2a buffers: [8 dst cores x 128 f, 512 tok] bf16
        self.a2a_in = [self.dram.tile([N_CORES * P, 512], BF16, tag=f"a2a_in{h}", name=f"a2a_in{h}")
                       for h in range(HEADS_PER_CORE)]
        self.a2a_out = [self.dram.tile([N_CORES * P, 512], BF16, tag=f"a2a_out{h}", name=f"a2a_out{h}")
                        for h in range(HEADS_PER_CORE)]


def wq_slice(cx, h, q):
    base = h * 2 * HEAD_DIM + q * HEAD_DIM
    return cx.wq_sb[:, base:base + HEAD_DIM]


def _rope(cx, ps, dst, tb):
    """dst = psum * cos + pairswap(psum) * signed_sin, written as bf16."""
    nc = cx.nc
    s0 = (tb % (S // 512)) * 512  # seq offset within batch
    shuf = cx.ropep.tile([P, 512], F32, tag="shuf", name="shuf")
    nc.vector.stream_shuffle(shuf[:], ps, SWAP_MASK)
    t1 = cx.ropep.tile([P, 512], F32, tag="t1", name="t1")
    nc.vector.tensor_mul(t1[:], ps, cx.cos_sb[:, s0:s0 + 512])
    t2 = cx.ropep.tile([P, 512], F32, tag="t2", name="t2")
    nc.vector.tensor_mul(t2[:], shuf[:], cx.sin_sb[:, s0:s0 + 512])
    nc.vector.tensor_add(dst, t1[:], t2[:])


def phase_proj(cx, psBig, psSmall, tbs):
    nc = cx.nc
    xT3 = cx.io["xT"][:].rearrange("(h p) t -> p h t", p=P)
    for tb in tbs:
        xt = cx.xtp.tile([P, HS * 512], BF16, tag="xt", name="xt")
        for hh in range(2):
            nc.sync.dma_start(
                out=xt[:, hh * 8 * 512:(hh + 1) * 8 * 512].rearrange("p (h t) -> p h t", h=8),
                in_=xT3[:, hh * 8:(hh + 1) * 8, tb * 512:(tb + 1) * 512],
            )
        qq_ps = psBig.tile([P, 1024], F32, tag="big", name="big")   # Q0 | Q1
        kv_ps = psBig.tile([P, 1024], F32, tag="big", name="big")   # K | V^T
        # all Q matmuls first: the opening matmul only waits on wq + half of x
        for h in range(HS):
            st, sp = h == 0, h == HS - 1
            xs = xt[:, h * 512:(h + 1) * 512]
            nc.tensor.matmul(qq_ps[:, 0:512], wq_slice(cx, h, 0), xs, start=st, stop=sp)
            nc.tensor.matmul(qq_ps[:, 512:1024], wq_slice(cx, h, 1), xs, start=st, stop=sp)
        for h in range(HS):
            st, sp = h == 0, h == HS - 1
            xs = xt[:, h * 512:(h + 1) * 512]
            nc.tensor.matmul(kv_ps[:, 0:512], cx.wk_sb[:, h * P:(h + 1) * P], xs, start=st, stop=sp)
            nc.tensor.matmul(kv_ps[:, 512:1024], cx.wv_sb[:, h * P:(h + 1) * P], xs, start=st, stop=sp)
        _rope(cx, qq_ps[:, 0:512], cx.QT[0][:, tb * 512:(tb + 1) * 512], tb)
        _rope(cx, qq_ps[:, 512:1024], cx.QT[1][:, tb * 512:(tb + 1) * 512], tb)
        _rope(cx, kv_ps[:, 0:512], cx.KT[:, tb * 512:(tb + 1) * 512], tb)
        # V: copy to bf16, transpose 128x128 blocks into Vaug ([tok, d])
        vt_sb = cx.smallp.tile([P, 512], BF16, tag="vtsb", name="vtsb")
        nc.vector.tensor_copy(vt_sb[:], kv_ps[:, 512:1024])
        for j in range(4):
            kbi = tb * 4 + j
            vtt = psSmall.tile([P, P], BF16, tag="small", name="small")
            nc.tensor.transpose(vtt[:], vt_sb[:, j * P:(j + 1) * P], cx.identity[:])
            nc.vector.tensor_copy(cx.Vaug[kbi][:, 0:HEAD_DIM], vtt[:])
            nc.vector.memset(cx.Vaug[kbi][:, HEAD_DIM:HEAD_DIM + 1], 1.0)


def phase_attention(cx, psBig, psSmall, hq, b):
    """Attention for local head hq, batch b; fills a2a_in[hq] dst blocks."""
    nc = cx.nc
    if True:
        if True:
            kb0 = b * (S // P)      # first key block of this batch
            q0 = b * S              # first token of this batch
            for qt in range(S // 512):
                i0 = qt * 4         # first q chunk (of 128) in this tile
                qsl = slice(q0 + qt * 512, q0 + (qt + 1) * 512)
                # expP view per kb: (tile, col offset of its 512-wide half)
                ep = []
                for g in range(qt * 2 + 2):
                    st_ps = psBig.tile([P, 1024], F32, tag="big", name="big")
                    e2 = cx.expp.tile([P, 1024], BF16, tag="expp", name="expp")
                    diag = g >= qt * 2
                    for half in range(2):
                        kb = g * 2 + half
                        m = max(kb - i0, 0)  # leading q-chunks of this tile never read
                        lo = half * 512
                        nc.tensor.matmul(
                            st_ps[:, lo + m * P:lo + 512],
                            cx.KT[:, (kb0 + kb) * P:(kb0 + kb + 1) * P],
                            cx.QT[hq][:, qsl.start + m * P:qsl.stop],
                            start=True, stop=True,
                        )
                        if diag:
                            nc.scalar.activation(e2[:, lo + m * P:lo + 512], st_ps[:, lo + m * P:lo + 512],
                                                 EXP, scale=SCALE)
                            nc.vector.tensor_mul(e2[:, lo + m * P:lo + (m + 1) * P],
                                                 e2[:, lo + m * P:lo + (m + 1) * P], cx.tri_sb[:])
                        ep.append((e2, lo))
                    if not diag:
                        nc.scalar.activation(e2[:], st_ps[:], EXP, scale=SCALE)
                # staged a2a block for this qtile: [128 f, 512 tok]
                ot_sb = cx.otp.tile([P, 512], BF16, tag="ot", name="ot")
                for c in range(4):
                    qb = i0 + c
                    oaug = psSmall.tile([P, HEAD_DIM + 1], F32, tag="small", name="small")
                    for kb in range(qb + 1):
                        et, off = ep[kb]
                        nc.tensor.matmul(
                            oaug[:],
                            et[:, off + c * P:off + (c + 1) * P],
                            cx.Vaug[kb0 + kb][:],
                            start=(kb == 0), stop=(kb == qb),
                        )
                    recip = cx.smallp.tile([P, 1], F32, tag="recip", name="recip")
                    nc.vector.reciprocal(recip[:], oaug[:, HEAD_DIM:HEAD_DIM + 1])
                    osb = cx.smallp.tile([P, P], BF16, tag="osb", name="osb")
                    nc.vector.tensor_scalar(osb[:], oaug[:, 0:HEAD_DIM], recip[:], None, mybir.AluOpType.mult)
                    ot = psSmall.tile([P, P], BF16, tag="small", name="small")
                    nc.tensor.transpose(ot[:], osb[:], cx.identity[:])
                    nc.vector.tensor_copy(ot_sb[:, c * P:(c + 1) * P], ot[:])
                dst = b * 4 + qt     # destination core for these 512 tokens
                nc.sync.dma_start(out=cx.a2a_in[hq][dst * P:(dst + 1) * P, :], in_=ot_sb[:])


def a2a(cx, hq):
    if cx.single_core:
        # timeline-sim variant: model the exchange as a local DRAM->DRAM copy
        cx.nc.sync.dma_start(out=cx.a2a_out[hq][:], in_=cx.a2a_in[hq][:])
        _load_attnT(cx, hq)
        return
    cx.nc.gpsimd.collective_compute(
        "AllToAll",
        mybir.AluOpType.bypass,
        replica_groups=[list(range(N_CORES))],
        ins=[cx.a2a_in[hq].opt()],
        outs=[cx.a2a_out[hq].opt()],
    )
    _load_attnT(cx, hq)


def _load_attnT(cx, hq):
    """One 3D DMA: a2a_out[hq] [8*128 f, 512 t] -> [128, 8*512] SBUF."""
    nc = cx.nc
    cx.attnT = getattr(cx, "attnT", [None] * HEADS_PER_CORE)
    t = cx.attntp.tile([P, N_CORES * 512], BF16, tag=f"attnt{hq}", name=f"attnt{hq}")
    nc.sync.dma_start(
        out=t[:].rearrange("p (c t) -> p c t", c=N_CORES),
        in_=cx.a2a_out[hq][:].rearrange("(c p) t -> p c t", p=P),
    )
    cx.attnT[hq] = t


def phase_oproj(cx, psBig, psSmall):
    nc = cx.nc
    # global head fs = local head fs%2 of core fs//2; its slice sits in
    # attnT[fs%2] columns [(fs//2)*512, +512).
    woT3 = cx.io["woT"][:].rearrange("(f p) o -> p f o", p=P)
    evens = [2 * c for c in range(N_CORES)]
    odds = [2 * c + 1 for c in range(N_CORES)]

    def mms(o_ps, tb4, wo_sb, fss, start, stop):
        for n, fs in enumerate(fss):
            base = (fs // 2) * 512 + tb4 * P
            lhsT = cx.attnT[fs % 2][:, base:base + P]
            nc.tensor.matmul(o_ps[:, 0:512], lhsT, wo_sb[:, fs * 512:(fs + 1) * 512],
                             start=start and n == 0, stop=stop and n == len(fss) - 1)

    def evict(o_ps, tb4, hb):
        o_sb = cx.outp.tile([P, 512], F32, tag="outsb", name="outsb")
        nc.vector.tensor_copy(o_sb[:], o_ps[:, 0:512])
        nc.sync.dma_start(out=cx.io["out"][tb4 * P:(tb4 + 1) * P, hb * 512:(hb + 1) * 512], in_=o_sb[:])

    # Software-pipelined accumulation over 16 (hb, tb4) groups with a
    # 5-deep evens-ahead window (3 big + 2 small PSUM slots): head-0
    # contributions keep the PE busy while the head-1 collective lands.
    wo_tiles = {}

    def ensure_wo(hb):
        if hb not in wo_tiles:
            w = cx.wop.tile([P, HS * 512], BF16, tag="wo", name="wo")
            nc.scalar.dma_start(
                out=w[:].rearrange("p (f o) -> p f o", f=HS),
                in_=woT3[:, :, hb * 512:(hb + 1) * 512],
            )
            wo_tiles[hb] = w
        return wo_tiles[hb]

    groups = [(hb, tb4) for hb in range(4) for tb4 in range(4)]
    DEPTH = 5
    tiles = {}

    def start_group(i):
        hb, tb4 = groups[i]
        wo_sb = ensure_wo(hb)
        if i % DEPTH < 3:
            t = psBig.tile([P, 1024], F32, tag="big", name="big")
        else:
            t = psSmall.tile([P, 512], F32, tag="small", name="small")
        tiles[i] = (t, wo_sb)
        mms(t, tb4, wo_sb, evens, True, False)

    for i in range(DEPTH):
        start_group(i)
    for i in range(len(groups)):
        hb, tb4 = groups[i]
        t, wo_sb = tiles.pop(i)
        mms(t, tb4, wo_sb, odds, False, True)
        evict(t, tb4, hb)
        if i + DEPTH < len(groups):
            start_group(i + DEPTH)


def emit_program(nc, nreps=1, single_core=False):
    io = {
        "xT": nc.dram_tensor("xT", [HIDDEN, T], BF16, kind="ExternalInput"),
        "wqT": nc.dram_tensor("wqT", [HIDDEN, HEADS_PER_CORE * HEAD_DIM], BF16, kind="ExternalInput"),
        "wkT": nc.dram_tensor("wkT", [HIDDEN, HEAD_DIM], BF16, kind="ExternalInput"),
        "wvT": nc.dram_tensor("wvT", [HIDDEN, HEAD_DIM], BF16, kind="ExternalInput"),
        "woT": nc.dram_tensor("woT", [HIDDEN, HIDDEN], BF16, kind="ExternalInput"),
        "cosT": nc.dram_tensor("cosT", [P, S], F32, kind="ExternalInput"),
        "sinTs": nc.dram_tensor("sinTs", [P, S], F32, kind="ExternalInput"),
        "trimask": nc.dram_tensor("trimask", [P, P], BF16, kind="ExternalInput"),
        "out": nc.dram_tensor("out", [T // N_CORES, HIDDEN], F32, kind="ExternalOutput"),
    }
    with tile.TileContext(nc) as tc:
        for _rep in range(nreps):
            with ExitStack() as es:
                cx = Ctx(tc, es, io, single_core=single_core)
                psBig = es.enter_context(tc.tile_pool(name="psBig", bufs=3, space="PSUM"))
                psSmall = es.enter_context(tc.tile_pool(name="psSmall", bufs=2, space="PSUM"))
                phase_proj(cx, psBig, psSmall, range(0, 4))
                phase_attention(cx, psBig, psSmall, 0, 0)
                phase_proj(cx, psBig, psSmall, range(4, 8))
                phase_attention(cx, psBig, psSmall, 0, 1)
                a2a(cx, 0)
                phase_attention(cx, psBig, psSmall, 1, 0)
                phase_attention(cx, psBig, psSmall, 1, 1)
                a2a(cx, 1)
                phase_oproj(cx, psBig, psSmall)


def build_program(nreps=1, single_core=False):
    if single_core:
        nc = bacc.Bacc("TRN2", target_bir_lowering=False, debug=False)
        emit_program(nc, nreps, single_core=True)
    else:
        nc = bacc.Bacc("TRN2", target_bir_lowering=False, debug=False, num_devices=N_CORES)
        emit_program(nc, nreps)
    nc.compile()
    return nc


def shard_inputs(x, Wq, Wk, Wv, Wo):
    x = np.asarray(x, dtype=np.float32)
    Wq = np.asarray(Wq, dtype=np.float32)
    Wk = np.asarray(Wk, dtype=np.float32)
    Wv = np.asarray(Wv, dtype=np.float32)
    Wo = np.asarray(Wo, dtype=np.float32)

    # x: [B,S,H] -> xT [H, B*S] (batch-major tokens)
    xT = np.ascontiguousarray(x.reshape(T, HIDDEN).T).astype(ml_dtypes.bfloat16)
    woT = np.ascontiguousarray(Wo.T).astype(ml_dtypes.bfloat16)

    # RoPE tables in [d, t] layout, sin pre-signed for the pair-swap trick
    j = np.arange(0, HEAD_DIM, 2, dtype=np.float32)
    inv_freq = 1.0 / (ROPE_BASE ** (j / HEAD_DIM))           # [64]
    pos = np.arange(S, dtype=np.float32)
    ang = inv_freq[:, None] * pos[None, :]                   # [64, S]
    cosT = np.repeat(np.cos(ang), 2, axis=0).astype(np.float32)   # [128, S]
    sin = np.sin(ang)
    sinTs = np.empty((HEAD_DIM, S), np.float32)
    sinTs[0::2] = -sin
    sinTs[1::2] = sin

    trimask = np.triu(np.ones((P, P), np.float32)).astype(ml_dtypes.bfloat16)

    in_maps = []
    for i in range(N_CORES):
        g = i // 2
        in_maps.append({
            "xT": xT,
            "wqT": np.ascontiguousarray(Wq[2 * i * HEAD_DIM:(2 * i + 2) * HEAD_DIM, :].T).astype(ml_dtypes.bfloat16),
            "wkT": np.ascontiguousarray(Wk[g * HEAD_DIM:(g + 1) * HEAD_DIM, :].T).astype(ml_dtypes.bfloat16),
            "wvT": np.ascontiguousarray(Wv[g * HEAD_DIM:(g + 1) * HEAD_DIM, :].T).astype(ml_dtypes.bfloat16),
            "woT": woT,
            "cosT": cosT,
            "sinTs": sinTs,
            "trimask": trimask,
        })
    return in_maps


_CACHED_NC = None


def kernel(x, Wq, Wk, Wv, Wo):
    global _CACHED_NC
    if _CACHED_NC is None:
        _CACHED_NC = build_program()
    nc = _CACHED_NC
    in_maps = shard_inputs(x, Wq, Wk, Wv, Wo)
    res = run_bass_kernel_spmd(nc, in_maps, core_ids=list(range(N_CORES)))
    outs = np.concatenate([res.results[i]["out"] for i in range(N_CORES)], axis=0)
    return outs.reshape(B, S, HIDDEN).astype(np.float32)
